# revision 3
# baseline (speedup 1.0000x reference)
"""BitLinear (BitNet 1.58-bit ternary) distributed Trainium2 kernel.

Reference semantics:
    scale = max(mean(|w|), 1e-5)
    w_q   = sign(w) * (|w| > scale/3)          # ternary {-1, 0, 1}
    out   = (x @ w_q.T) * scale                # x: [4, 2048, 2048], w: [2048, 2048]

Sharding: data-parallel over tokens (1024 of 8192 per core), weight
replicated; each core computes the scale locally, so there are no
collectives (cross-core sync points absorb the harness' launch skew,
and a scalar AllReduce has a ~20us floor -- as long as the 8-MiB
scale pass it would replace).

Host-side prep: transpose w to [in, out] and cast to fp16 with a
threshold "nudge": the handful of elements whose fp16 rounding would
flip the |w| > scale/3 comparison (or that sit within 5e-5 of the
threshold) are moved one fp16 ulp so the fp16 copy classifies exactly
like the f32 original, robust to ~1e-5 wobble in the device-computed
mean. fp16 rounding is unbiased, so the device mean matches the f32
mean to ~1e-7 relative. The f32 weight is never shipped; per-core HBM
traffic is 24 MiB (2x8 wh + 4 x + 4 out bf16).

Device schedule (single HWDGE ring, program-ordered; pass-2/x DMAs
need no gating -- FIFO position does it):
  pass 1 (~24 us, DMA-bound): stream 8 wh 1-MiB pair-tiles; |w| sums
          alternate ACT (in-place Abs + accum_out, 3.7us) and DVE
          (reduce XY, 4.4us) so each engine sees a 5.8us period and
          the stream runs at DMA pace; the last pair splits across
          both engines to halve the serial tail. Dummy bf16 matmul
          ladders (data-gated on the reduces) keep the PE's HAM
          clock-gate warm so phase-1 matmuls run at 2.4 GHz.
  scale:  sum partials, broadcast via ones-matmul, t = s/3.
  pass 2: re-stream wh; each pair is quantized column-split across
          DVE (comparisons), ACT (signs), and GpSimd (combines) so
          production (~2.9us/pair) tracks DMA and no engine saturates.
  x m0 lands between the passes, m1 after pair 0; the k-outer phase
  starts ~38us in and is PE-bound from there.

Quantization: ternary, computed doubled:
  DVE cols:  wq2 = 2*(w > t) - 2*(w < -t)              in {-2, 0, 2}
  ACT cols:  wq2 = Sign(w + t) + Sign(w - t)           in {-2, 0, 2}
The missing 1/2 is folded into the output scaling (psum * scale/2).

Matmul: bf16 x bf16 -> fp32 PSUM, K=2048 contracted in 16 accumulating
matmuls, N=512 per PSUM bank. The first two m-tiles run k-outer across
8 PSUM banks chasing the quant stream; the remaining six m-tiles run
as clean dense passes (~14us each, ~97% of the warm-PE roofline).
Output is written bf16 (upcast on host), halving the store traffic.
"""

import sys

sys.path.insert(0, "/opt/trn_rl_repo")

import numpy as np

N_CORES = 8
B, S, D = 4, 2048, 2048        # x: [B, S, D]
OUT = 2048                     # out_features
TOK = B * S                    # 8192 tokens
TPC = TOK // N_CORES           # 1024 tokens per core
KT = D // 128                  # 16 K-tiles of 128
MT = TPC // 128                # 8 M-tiles per core
NT = OUT // 512                # 4 N-tiles of 512
N_ELEM = float(D * OUT)        # elements of w
EPS = 1e-5
M_P1 = 2                       # m-tiles in the k-outer first phase
HQ = OUT // 2                  # column split point for quant


def build_kernel():
    from concourse import bacc, tile, mybir

    f32 = mybir.dt.float32
    bf16 = mybir.dt.bfloat16
    fp16 = mybir.dt.float16
    Alu = mybir.AluOpType
    Act = mybir.ActivationFunctionType
    X = mybir.AxisListType.X

    nc = bacc.Bacc(None, target_bir_lowering=False)
    x_ext = nc.declare_dram_parameter("x", [TPC, D], bf16, isOutput=False)
    wh_ext = nc.declare_dram_parameter("wh", [D, OUT], fp16, isOutput=False)
    out_ext = nc.declare_dram_parameter("out", [TPC, OUT], bf16, isOutput=True)

    with tile.TileContext(nc) as tc:
        with (
            tc.tile_pool(name="persist", bufs=1) as persist,
            tc.tile_pool(name="whf", bufs=4) as whf_pool,
            tc.tile_pool(name="xbuf", bufs=8) as xbuf_pool,
            tc.tile_pool(name="sgn", bufs=6) as sgn_pool,
            tc.tile_pool(name="outp", bufs=2) as out_pool,
            tc.tile_pool(name="psum", bufs=8, space="PSUM") as psum_pool,
        ):
            wq = persist.tile([128, KT, OUT], bf16)      # quantized w^T (doubled)
            ones = persist.tile([128, 128], f32)
            ones_bf = persist.tile([128, 128], bf16)
            dummy_rhs = persist.tile([128, 512], bf16)
            partials_a = persist.tile([128, 5], f32)     # ACT accum_out cols
            partials_d = persist.tile([128, 4], f32)     # DVE reduce cols
            tot_a = persist.tile([128, 1], f32)
            tot = persist.tile([128, 1], f32)
            scale_sb = persist.tile([128, 1], f32)
            t_pos = persist.tile([128, 1], f32)
            t_neg = persist.tile([128, 1], f32)
            s_half = persist.tile([128, 1], f32)

            nc.vector.memset(ones[:], 1.0)
            nc.vector.memset(ones_bf[:], 1.0)
            nc.vector.memset(dummy_rhs[:], 1.0)
            # PE warm-up: fetch PE's IRAM block + park the sequencer early
            warm = psum_pool.tile([128, 512], f32, tag="psum", name="warm")
            nc.tensor.matmul(
                warm[:, 0:1], ones[:], ones[:, 0:1], start=True, stop=True
            )

            def keep_warm(gate_src, n_mm):
                # data-gated dummy matmuls: fire right after gate_src is
                # written, keeping the PE HAM clock-gate warm through the
                # prefix (PE is otherwise idle until the scale lands)
                nc.vector.tensor_copy(dummy_rhs[:, 0:1], gate_src)
                kw = psum_pool.tile([128, 512], f32, tag="psum", name="kw")
                for _ in range(n_mm):
                    nc.tensor.matmul(
                        kw[:], ones_bf[:], dummy_rhs[:], start=True, stop=True
                    )

            def x_dma(m):
                xb = xbuf_pool.tile([128, KT, 128], bf16, tag="xbuf", name=f"xb{m}")
                nc.sync.dma_start(
                    xb[:],
                    x_ext[m * 128 : (m + 1) * 128, :].rearrange(
                        "p (k c) -> p k c", k=KT
                    ),
                )
                return xb

            def wh_dma(j, name):
                wh = whf_pool.tile([128, 2, OUT], fp16, tag="whf", name=name)
                nc.sync.dma_start(
                    wh[:],
                    wh_ext[j * 256 : (j + 1) * 256, :].rearrange(
                        "(t p) o -> p t o", p=128
                    ),
                )
                return wh

            # ---- pass 1: stream wh, abs-sums alternating ACT/DVE ----
            for j in range(KT // 2):
                wh = wh_dma(j, f"wh{j}")
                if j == KT // 2 - 1:
                    # split the last pair across both engines (serial tail)
                    nc.scalar.activation(
                        wh[:, 0, :], wh[:, 0, :], Act.Abs,
                        accum_out=partials_a[:, 4:5],
                    )
                    nc.vector.tensor_reduce(
                        partials_d[:, 3:4], wh[:, 1, :],
                        axis=X, op=Alu.add, apply_absolute_value=True,
                    )
                elif j % 2 == 0:
                    nc.scalar.activation(
                        wh[:], wh[:], Act.Abs,
                        accum_out=partials_a[:, j // 2 : j // 2 + 1],
                    )
                else:
                    nc.vector.tensor_reduce(
                        partials_d[:, j // 2 : j // 2 + 1], wh[:],
                        axis=mybir.AxisListType.XY,
                        op=Alu.add, apply_absolute_value=True,
                    )
                if j == 3:
                    keep_warm(partials_d[:, 1:2], 8)
                elif j == 4:
                    keep_warm(partials_a[:, 2:3], 8)
                elif j == 5:
                    keep_warm(partials_d[:, 2:3], 8)
                elif j == 6:
                    keep_warm(partials_a[:, 3:4], 8)

            keep_warm(partials_a[:, 4:5], 4)

            # ---- scale: sum partials, broadcast via ones-matmul ----
            nc.vector.tensor_reduce(tot_a[:], partials_a[:], axis=X, op=Alu.add)
            nc.vector.tensor_reduce(tot[:], partials_d[:], axis=X, op=Alu.add)
            nc.vector.tensor_tensor(tot[:], tot[:], tot_a[:], Alu.add)
            pbc = psum_pool.tile([128, 512], f32, tag="psum", name="pbc")
            nc.tensor.matmul(pbc[:, 0:1], ones[:], tot[:], start=True, stop=True)
            nc.vector.tensor_scalar(
                scale_sb[:], pbc[:, 0:1], 1.0 / N_ELEM, EPS, Alu.mult, Alu.max
            )
            nc.vector.tensor_scalar(t_pos[:], scale_sb[:], 1.0 / 3.0, None, Alu.mult)
            nc.vector.tensor_scalar(t_neg[:], scale_sb[:], -1.0 / 3.0, None, Alu.mult)
            nc.vector.tensor_scalar(s_half[:], scale_sb[:], 0.5, None, Alu.mult)

            # ---- quantize one K-tile, columns split across engines:
            # DVE does [0:HQ] (2 compares + combine), ACT does [HQ:] (2
            # signs), GpSimd combines the ACT half. ----
            def quantize(k, src):
                neg = sgn_pool.tile([128, HQ], bf16, tag="sgn", name=f"n{k}")
                nc.vector.tensor_scalar(
                    wq[:, k, :HQ], src[:, :HQ], t_pos[:, 0:1], 2.0,
                    Alu.is_gt, Alu.mult,
                )
                nc.vector.tensor_scalar(
                    neg[:], src[:, :HQ], t_neg[:, 0:1], -2.0, Alu.is_lt, Alu.mult
                )
                nc.vector.tensor_tensor(
                    wq[:, k, :HQ], wq[:, k, :HQ], neg[:], Alu.add
                )
                s1 = sgn_pool.tile([128, HQ], bf16, tag="sgn", name=f"s1_{k}")
                s2 = sgn_pool.tile([128, HQ], bf16, tag="sgn", name=f"s2_{k}")
                nc.scalar.activation(s1[:], src[:, HQ:], Act.Sign, bias=t_pos[:, 0:1])
                nc.scalar.activation(s2[:], src[:, HQ:], Act.Sign, bias=t_neg[:, 0:1])
                nc.gpsimd.tensor_tensor(wq[:, k, HQ:], s1[:], s2[:], Alu.add)

            # ---- pass 2 + x: re-stream wh (slots self-gate on pass-1
            # reduces), quantize each pair at DMA pace ----
            xbufs = {0: x_dma(0)}
            for j in range(KT // 2):
                wh = wh_dma(j, f"whb{j}")
                quantize(2 * j, wh[:, 0, :])
                quantize(2 * j + 1, wh[:, 1, :])
                if j == 0:
                    xbufs[1] = x_dma(1)
            for m in range(M_P1, MT):
                xbufs[m] = x_dma(m)

            # ---- matmul: out[m,n] = sum_k x[k,m].T @ wq[k,n] ----
            def do_mtile(ms):
                psums = [
                    psum_pool.tile([128, 512], f32, tag="psum", name=f"ps{i}")
                    for i in range(NT * len(ms))
                ]
                for ki, k in enumerate(range(KT)):
                    for mi, m in enumerate(ms):
                        for n in range(NT):
                            nc.tensor.matmul(
                                psums[mi * NT + n][:],
                                xbufs[m][:, k, :],
                                wq[:, k, n * 512 : (n + 1) * 512],
                                start=(ki == 0),
                                stop=(ki == KT - 1),
                            )
                for mi, m in enumerate(ms):
                    ot = out_pool.tile([128, OUT], bf16, tag="outp", name=f"ot{m}")
                    for n in range(NT):
                        nc.scalar.activation(
                            ot[:, n * 512 : (n + 1) * 512],
                            psums[mi * NT + n][:],
                            Act.Copy,
                            scale=s_half[:, 0:1],
                        )
                        nc.sync.dma_start(
                            out_ext[m * 128 : (m + 1) * 128, n * 512 : (n + 1) * 512],
                            ot[:, n * 512 : (n + 1) * 512],
                        )

            do_mtile(list(range(M_P1)))
            for m in range(M_P1, MT):
                do_mtile([m])

    nc.finalize()
    return nc


def _nudged_fp16(weight):
    """fp16 copy of w^T whose |.| > mean(|.|)/3 classification matches the
    f32 original exactly, with >=1-ulp clearance from the threshold."""
    wT = np.ascontiguousarray(weight.T).astype(np.float32)
    t64 = max(np.abs(wT).astype(np.float64).mean(), EPS) / 3.0
    big_ref = np.abs(wT).astype(np.float64) > t64
    wh = wT.astype(np.float16)
    sgn = np.where(wT < 0, np.float16(-1), np.float16(1))
    for _ in range(4):
        a = np.abs(wh.astype(np.float64))
        t = max(a.mean(), EPS) / 3.0
        band = 5e-5 * t
        bad_big = big_ref & (a <= t + band)
        bad_small = (~big_ref) & (a >= t - band)
        if not (bad_big.any() or bad_small.any()):
            break
        aa = np.abs(wh)
        aa[bad_big] = np.nextafter(aa[bad_big], np.float16(np.inf))
        aa[bad_small] = np.nextafter(aa[bad_small], np.float16(0))
        wh = aa * sgn
    return wh


_NC_CACHE = None


def kernel(x, weight):
    global _NC_CACHE
    import ml_dtypes
    from concourse.bass_utils import run_bass_kernel_spmd

    x = np.asarray(x, dtype=np.float32).reshape(TOK, D)
    weight = np.asarray(weight, dtype=np.float32)
    wh = _nudged_fp16(weight)                                # [in, out] fp16
    in_maps = []
    for i in range(N_CORES):
        shard_t = x[i * TPC : (i + 1) * TPC].T                      # [in, tok]
        tiled = (
            shard_t.reshape(KT, 128, MT, 128)
            .transpose(2, 1, 0, 3)
            .reshape(MT * 128, KT * 128)
        )
        in_maps.append(
            {"x": np.ascontiguousarray(tiled).astype(ml_dtypes.bfloat16),
             "wh": wh}
        )

    if _NC_CACHE is None:
        _NC_CACHE = build_kernel()
    res = run_bass_kernel_spmd(_NC_CACHE, in_maps, core_ids=list(range(N_CORES)))
    outs = [np.asarray(res.results[i]["out"]) for i in range(N_CORES)]
    return np.concatenate(outs, axis=0).reshape(B, S, OUT).astype(np.float32)


# revision 4
# speedup vs baseline: 1.1384x; 1.1384x over previous
"""BitLinear (BitNet 1.58-bit ternary) distributed Trainium2 kernel.

Reference semantics:
    scale = max(mean(|w|), 1e-5)
    w_q   = sign(w) * (|w| > scale/3)          # ternary {-1, 0, 1}
    out   = (x @ w_q.T) * scale                # x: [4, 2048, 2048], w: [2048, 2048]

Sharding: data-parallel over tokens (1024 of 8192 per core), weight
replicated; each core computes the scale locally, so there are no
collectives (a scalar AllReduce has a ~20us floor -- as long as the
8-MiB scale pass it would replace -- and cross-core sync absorbs
launch skew).

Host-side prep: transpose w to [in, out] and cast to fp16 with a
threshold "nudge": elements whose fp16 rounding would flip the
|w| > scale/3 comparison (or that sit within 5e-5 of the threshold)
are moved one fp16 ulp so the fp16 copy classifies exactly like the
f32 original, robust to ~1e-5 wobble in the device-computed mean.
The f32 weight is never shipped; per-core traffic is ~18 MiB.

Device schedule (single HWDGE ring, program-ordered):
  pass 1 (~24us, DMA-bound): stream wh as 16 half-MiB k-tiles;
          |w| sums alternate ACT (Abs + accum_out, 1.9us) and DVE
          (reduce X, 2.2us) so each engine sees a 2.6us period and
          the stream runs at DMA pace. The last tile splits across
          both engines (serial tail ~1us). Tiles 12-15 stay resident
          in the pool (12 bufs); 12/14 reduce via a scratch dest so
          their values survive (in-place Abs would destroy signs).
          Dummy bf16 matmul ladders (data-gated on the ACT partials)
          keep the PE's HAM clock-gate warm through the prefix.
  scale:  one 18-col partials reduce, ones-matmul broadcast, t = s/3.
  quant:  starts at the scale on the RESIDENT tiles k=12..15 (zero
          arrival latency), then chases the k=0..11 re-stream. Paths
          alternate: k odd = ACT (2 Signs) + DVE add; k even = DVE
          (2 fused compares + add). ~2.0us/tile production, no GpSimd
          (concurrent GpSimd ops degrade DVE throughput ~5x).
  x m0/m1 land right behind pass 1; the k-outer phase-1 (m0,m1 across
  8 PSUM banks, k order 12..15,0..11) starts ~34us in; the six dense
  m-tiles follow at ~14us each (~97% of warm-PE roofline).

Quantization: ternary, computed doubled:
  DVE tiles: wq2 = 2*(w > t) - 2*(w < -t)              in {-2, 0, 2}
  ACT tiles: wq2 = Sign(w + t) + Sign(w - t)           in {-2, 0, 2}
The missing 1/2 is folded into the output scaling (psum * scale/2).
Output is written bf16 (upcast on host), halving store traffic.
"""

import sys

sys.path.insert(0, "/opt/trn_rl_repo")

import numpy as np

N_CORES = 8
B, S, D = 4, 2048, 2048        # x: [B, S, D]
OUT = 2048                     # out_features
TOK = B * S                    # 8192 tokens
TPC = TOK // N_CORES           # 1024 tokens per core
KT = D // 128                  # 16 K-tiles of 128
MT = TPC // 128                # 8 M-tiles per core
NT = OUT // 512                # 4 N-tiles of 512
N_ELEM = float(D * OUT)        # elements of w
EPS = 1e-5
M_P1 = 2                       # m-tiles in the k-outer first phase
N_RES = 4                      # trailing k-tiles quantized from residency
K_ORDER = list(range(KT - N_RES, KT)) + list(range(KT - N_RES))


def build_kernel():
    from concourse import bacc, tile, mybir

    f32 = mybir.dt.float32
    bf16 = mybir.dt.bfloat16
    fp16 = mybir.dt.float16
    Alu = mybir.AluOpType
    Act = mybir.ActivationFunctionType
    X = mybir.AxisListType.X

    nc = bacc.Bacc(None, target_bir_lowering=False)
    x_ext = nc.declare_dram_parameter("x", [TPC, D], bf16, isOutput=False)
    wh_ext = nc.declare_dram_parameter("wh", [D, OUT], fp16, isOutput=False)
    out_ext = nc.declare_dram_parameter("out", [TPC, OUT], bf16, isOutput=True)

    with tile.TileContext(nc) as tc:
        with (
            tc.tile_pool(name="persist", bufs=1) as persist,
            tc.tile_pool(name="whf", bufs=12) as whf_pool,
            tc.tile_pool(name="scr", bufs=2) as scr_pool,
            tc.tile_pool(name="xbuf", bufs=8) as xbuf_pool,
            tc.tile_pool(name="sgn", bufs=4) as sgn_pool,
            tc.tile_pool(name="outp", bufs=2) as out_pool,
            tc.tile_pool(name="psum", bufs=8, space="PSUM") as psum_pool,
        ):
            wq = persist.tile([128, KT, OUT], bf16)      # quantized w^T (doubled)
            ones = persist.tile([128, 128], f32)
            ones_bf = persist.tile([128, 128], bf16)
            dummy_rhs = persist.tile([128, 512], bf16)
            partials = persist.tile([128, KT + 2], f32)
            tot = persist.tile([128, 1], f32)
            scale_sb = persist.tile([128, 1], f32)
            t_pos = persist.tile([128, 1], f32)
            t_neg = persist.tile([128, 1], f32)
            s_half = persist.tile([128, 1], f32)

            nc.vector.memset(ones[:], 1.0)
            nc.vector.memset(ones_bf[:], 1.0)
            nc.vector.memset(dummy_rhs[:], 1.0)
            nc.vector.memset(partials[:], 0.0)
            # PE warm-up: fetch PE's IRAM block + park the sequencer early
            warm = psum_pool.tile([128, 512], f32, tag="psum", name="warm")
            nc.tensor.matmul(
                warm[:, 0:1], ones[:], ones[:, 0:1], start=True, stop=True
            )

            def keep_warm(gate_src, n_mm):
                # data-gated dummy matmuls: fire right after gate_src is
                # written, keeping the PE HAM clock-gate warm through the
                # prefix (PE is otherwise idle until the scale lands)
                nc.vector.tensor_copy(dummy_rhs[:, 0:1], gate_src)
                kw = psum_pool.tile([128, 512], f32, tag="psum", name="kw")
                for _ in range(n_mm):
                    nc.tensor.matmul(
                        kw[:], ones_bf[:], dummy_rhs[:], start=True, stop=True
                    )

            def x_dma(m):
                xb = xbuf_pool.tile([128, KT, 128], bf16, tag="xbuf", name=f"xb{m}")
                nc.sync.dma_start(
                    xb[:],
                    x_ext[m * 128 : (m + 1) * 128, :].rearrange(
                        "p (k c) -> p k c", k=KT
                    ),
                )
                return xb

            def wh_dma(k, name):
                wh = whf_pool.tile([128, OUT], fp16, tag="whf", name=name)
                nc.sync.dma_start(wh[:], wh_ext[k * 128 : (k + 1) * 128, :])
                return wh

            # ---- pass 1: stream wh as 16 half-MiB k-tiles, |w| sums
            # alternating ACT/DVE at DMA pace ----
            wh_res = {}
            for k in range(KT):
                wh = wh_dma(k, f"wh{k}")
                if k >= KT - N_RES:
                    wh_res[k] = wh
                if k == KT - 1:
                    # split the last tile across both engines (serial tail);
                    # non-destructive on both paths (tile is resident)
                    H = OUT // 2
                    scr = scr_pool.tile([128, H], fp16, tag="scr", name="scrF")
                    nc.scalar.activation(
                        scr[:], wh[:, :H], Act.Abs,
                        accum_out=partials[:, KT : KT + 1],
                    )
                    nc.vector.tensor_reduce(
                        partials[:, KT + 1 : KT + 2], wh[:, H:],
                        axis=X, op=Alu.add, apply_absolute_value=True,
                    )
                elif k % 2 == 0:
                    if k >= KT - N_RES:
                        # resident tile: Abs to scratch so values survive
                        scr = scr_pool.tile([128, OUT], fp16, tag="scr",
                                            name=f"scr{k}")
                        nc.scalar.activation(
                            scr[:], wh[:], Act.Abs,
                            accum_out=partials[:, k : k + 1],
                        )
                    else:
                        nc.scalar.activation(
                            wh[:], wh[:], Act.Abs,
                            accum_out=partials[:, k : k + 1],
                        )
                else:
                    nc.vector.tensor_reduce(
                        partials[:, k : k + 1], wh[:],
                        axis=X, op=Alu.add, apply_absolute_value=True,
                    )
                if k in (8, 10, 12, 14):
                    keep_warm(partials[:, k : k + 1], 6)

            # ---- scale: sum partials, broadcast via ones-matmul ----
            nc.vector.tensor_reduce(tot[:], partials[:], axis=X, op=Alu.add)
            pbc = psum_pool.tile([128, 512], f32, tag="psum", name="pbc")
            nc.tensor.matmul(pbc[:, 0:1], ones[:], tot[:], start=True, stop=True)
            nc.vector.tensor_scalar(
                scale_sb[:], pbc[:, 0:1], 1.0 / N_ELEM, EPS, Alu.mult, Alu.max
            )
            nc.vector.tensor_scalar(t_pos[:], scale_sb[:], 1.0 / 3.0, None, Alu.mult)
            nc.vector.tensor_scalar(t_neg[:], scale_sb[:], -1.0 / 3.0, None, Alu.mult)
            nc.vector.tensor_scalar(s_half[:], scale_sb[:], 0.5, None, Alu.mult)

            # ---- quantize one K-tile: ACT path (k odd) or DVE path ----
            def quantize(k, src):
                if k % 2 == 1:
                    s1 = sgn_pool.tile([128, OUT], bf16, tag="sgn", name=f"s1_{k}")
                    s2 = sgn_pool.tile([128, OUT], bf16, tag="sgn", name=f"s2_{k}")
                    nc.scalar.activation(s1[:], src[:], Act.Sign, bias=t_pos[:, 0:1])
                    nc.scalar.activation(s2[:], src[:], Act.Sign, bias=t_neg[:, 0:1])
                    nc.vector.tensor_tensor(wq[:, k, :], s1[:], s2[:], Alu.add)
                else:
                    neg = sgn_pool.tile([128, OUT], bf16, tag="sgn", name=f"n{k}")
                    nc.vector.tensor_scalar(
                        wq[:, k, :], src[:], t_pos[:, 0:1], 2.0, Alu.is_gt, Alu.mult
                    )
                    nc.vector.tensor_scalar(
                        neg[:], src[:], t_neg[:, 0:1], -2.0, Alu.is_lt, Alu.mult
                    )
                    nc.vector.tensor_tensor(
                        wq[:, k, :], wq[:, k, :], neg[:], Alu.add
                    )

            # resident tail first: production starts at the scale with no
            # arrival latency
            for k in range(KT - N_RES, KT):
                quantize(k, wh_res[k])

            # ---- x m0/m1 + re-stream k=0..11, quantized at DMA pace ----
            xbufs = {0: x_dma(0), 1: x_dma(1)}
            for k in range(KT - N_RES):
                wh = wh_dma(k, f"whb{k}")
                quantize(k, wh)
            for m in range(M_P1, MT):
                xbufs[m] = x_dma(m)

            # ---- matmul: out[m,n] = sum_k x[k,m].T @ wq[k,n] ----
            def do_mtile(ms, korder):
                psums = [
                    psum_pool.tile([128, 512], f32, tag="psum", name=f"ps{i}")
                    for i in range(NT * len(ms))
                ]
                for ki, k in enumerate(korder):
                    for mi, m in enumerate(ms):
                        for n in range(NT):
                            nc.tensor.matmul(
                                psums[mi * NT + n][:],
                                xbufs[m][:, k, :],
                                wq[:, k, n * 512 : (n + 1) * 512],
                                start=(ki == 0),
                                stop=(ki == KT - 1),
                            )
                for mi, m in enumerate(ms):
                    ot = out_pool.tile([128, OUT], bf16, tag="outp", name=f"ot{m}")
                    for n in range(NT):
                        nc.scalar.activation(
                            ot[:, n * 512 : (n + 1) * 512],
                            psums[mi * NT + n][:],
                            Act.Copy,
                            scale=s_half[:, 0:1],
                        )
                        nc.sync.dma_start(
                            out_ext[m * 128 : (m + 1) * 128, n * 512 : (n + 1) * 512],
                            ot[:, n * 512 : (n + 1) * 512],
                        )

            do_mtile(list(range(M_P1)), K_ORDER)
            for m in range(M_P1, MT):
                do_mtile([m], list(range(KT)))

    nc.finalize()
    return nc


def _nudged_fp16(weight):
    """fp16 copy of w^T whose |.| > mean(|.|)/3 classification matches the
    f32 original exactly, with >=1-ulp clearance from the threshold."""
    wT = np.ascontiguousarray(weight.T).astype(np.float32)
    t64 = max(np.abs(wT).astype(np.float64).mean(), EPS) / 3.0
    big_ref = np.abs(wT).astype(np.float64) > t64
    wh = wT.astype(np.float16)
    sgn = np.where(wT < 0, np.float16(-1), np.float16(1))
    for _ in range(4):
        a = np.abs(wh.astype(np.float64))
        t = max(a.mean(), EPS) / 3.0
        band = 5e-5 * t
        bad_big = big_ref & (a <= t + band)
        bad_small = (~big_ref) & (a >= t - band)
        if not (bad_big.any() or bad_small.any()):
            break
        aa = np.abs(wh)
        aa[bad_big] = np.nextafter(aa[bad_big], np.float16(np.inf))
        aa[bad_small] = np.nextafter(aa[bad_small], np.float16(0))
        wh = aa * sgn
    return wh


_NC_CACHE = None


def kernel(x, weight):
    global _NC_CACHE
    import ml_dtypes
    from concourse.bass_utils import run_bass_kernel_spmd

    x = np.asarray(x, dtype=np.float32).reshape(TOK, D)
    weight = np.asarray(weight, dtype=np.float32)
    wh = _nudged_fp16(weight)                                # [in, out] fp16
    in_maps = []
    for i in range(N_CORES):
        shard_t = x[i * TPC : (i + 1) * TPC].T                      # [in, tok]
        tiled = (
            shard_t.reshape(KT, 128, MT, 128)
            .transpose(2, 1, 0, 3)
            .reshape(MT * 128, KT * 128)
        )
        in_maps.append(
            {"x": np.ascontiguousarray(tiled).astype(ml_dtypes.bfloat16),
             "wh": wh}
        )

    if _NC_CACHE is None:
        _NC_CACHE = build_kernel()
    res = run_bass_kernel_spmd(_NC_CACHE, in_maps, core_ids=list(range(N_CORES)))
    outs = [np.asarray(res.results[i]["out"]) for i in range(N_CORES)]
    return np.concatenate(outs, axis=0).reshape(B, S, OUT).astype(np.float32)
